# revision 11
# baseline (speedup 1.0000x reference)
"""Trainium2 Bass kernel for nn_AttnBlock: GroupNorm -> single-head spatial
self-attention (QKV 1x1 convs, softmax over 1024 positions, AV) -> proj 1x1
conv -> residual.

Sharding: data-parallel over batch. B=16 full batches -> 2 per NeuronCore x 8.
Each core runs an identical NEFF on its own batch shard; the host gathers.

Per-core layout (per batch, C=512 channels, N=1024 spatial positions):
  x, h, q, k: [C_part, N]  as 4 SBUF tiles [128, 1024]
  vT:         [N_part, C]  as 8 SBUF tiles [128, 512]   (v transposed "for free"
              by using h as the matmul's stationary operand)
  scoresT:    [P_key_part, p_query] tiles -> softmax denominator done with an
              all-ones stationary matmul (gives denom broadcast across
              partitions, exactly the layout the normalization multiply needs),
              so the whole attention pipeline needs zero transposes.

Matmuls use dtype float32r (fp32 bits, reduced-precision PE path): 1 cycle/row
vs fp32's 4 when the moving free dim >= 256. Accumulation stays fp32 in PSUM.

GroupNorm group reduction (16 channels/group on partitions) uses a tiny fp32
matmul against a block-diagonal averaging matrix, which also broadcasts the
group stats back to all 128 partitions in the same instruction.
rstd = sqrt(reciprocal(var+eps)) with the reciprocal on DVE, keeping ACT
table-set switches to sqrt<->exp boundaries (2 per batch).
"""

import os
import sys

import numpy as np

for _p in ("/opt/trn_rl_repo", "/root/.axon_site/_ro/trn_rl_repo"):
    if os.path.isdir(_p) and _p not in sys.path:
        sys.path.insert(0, _p)

import concourse.bacc as bacc
import concourse.tile as tile
import concourse.mybir as mybir
from concourse.alu_op_type import AluOpType
from concourse.bass_utils import run_bass_kernel_spmd

B, C, H, W = 16, 512, 32, 32
N = H * W                  # 1024 spatial positions
GROUPS = 32
GS = C // GROUPS           # 16 channels per group
NCORES = 8
BPC = B // NCORES          # batches per core
CT = C // 128              # channel partition-tiles
NT = N // 128              # position partition-tiles
NCH = N // 512             # 512-wide free chunks
EPS = 1e-5
ATTN_SCALE = float(C) ** -0.5

F32 = mybir.dt.float32
F32R = mybir.dt.float32r
Act = mybir.ActivationFunctionType

LAST_RESULTS = None        # BassKernelResults of the most recent run (for test.py)

_PROGRAM_CACHE = {}


def _build_program(flags, loop_reps=None):
    """Build the per-core Bass program. flags = (qb_nz, kb_nz, vb_nz, pb_nz).

    loop_reps: if set, wrap the whole per-core body in a hardware For_i loop
    executing it that many times (benchmarking only — output is identical
    every iteration since the program re-reads xs)."""
    qb_nz, kb_nz, vb_nz, pb_nz = flags
    nc = bacc.Bacc(
        "TRN2",
        target_bir_lowering=False,
        debug=False,
        enable_asserts=False,
        num_devices=NCORES,
    )

    def din(name, shape, dt=F32):
        return nc.dram_tensor(name, shape, dt, kind="ExternalInput").ap()

    xs = din("xs", [BPC, CT, 128, N])
    # weights feed fp32r matmuls; declared fp32r so the DMA is same-dtype
    # (numpy representation of float32r is float32)
    wq = din("wqT", [CT, 128, C], F32R)
    wk = din("wkT", [CT, 128, C], F32R)
    wv = din("wvT", [CT, 128, C], F32R)
    wp = din("wpT", [CT, 128, C], F32R)
    gnw = din("gnw", [CT, 128, 1])
    gnb = din("gnb", [CT, 128, 1])
    ones_d = din("ones", [128, 128], F32R)
    gmat_d = din("gmat", [128, 128])
    qb_d = din("qb", [CT, 128, 1]) if qb_nz else None
    kb_d = din("kb", [CT, 128, 1]) if kb_nz else None
    vb_d = din("vb", [128, C]) if vb_nz else None
    pb_d = din("pb", [CT, 128, 1]) if pb_nz else None

    out_d = nc.dram_tensor("out", [BPC, CT, 128, N], F32, kind="ExternalOutput").ap()

    with tile.TileContext(nc) as tc:
        _emit(tc, xs, wq, wk, wv, wp, gnw, gnb, ones_d, gmat_d,
              qb_d, kb_d, vb_d, pb_d, out_d, loop_reps=loop_reps)
    nc.compile()
    return nc


def _emit(tc, xs, wq, wk, wv, wp, gnw, gnb, ones_d, gmat_d,
          qb_d, kb_d, vb_d, pb_d, out_d, loop_reps=None):
    nc = tc.nc
    from contextlib import ExitStack
    ctx = ExitStack()
    with ctx:
        consts = ctx.enter_context(tc.tile_pool(name="consts", bufs=1))
        xin = ctx.enter_context(tc.tile_pool(name="xin", bufs=5))
        scr = ctx.enter_context(tc.tile_pool(name="scr", bufs=1))
        small = ctx.enter_context(tc.tile_pool(name="small", bufs=8))
        hpool = ctx.enter_context(tc.tile_pool(name="hpool", bufs=4))
        qpool = ctx.enter_context(tc.tile_pool(name="qpool", bufs=8))
        kpool = ctx.enter_context(tc.tile_pool(name="kpool", bufs=4))
        vpool = ctx.enter_context(tc.tile_pool(name="vpool", bufs=8))
        apool = ctx.enter_context(tc.tile_pool(name="apool", bufs=8))
        rpool = ctx.enter_context(tc.tile_pool(name="rpool", bufs=2))
        h2pool = ctx.enter_context(tc.tile_pool(name="h2pool", bufs=4))
        psmain = ctx.enter_context(tc.tile_pool(name="psmain", bufs=5, space="PSUM"))
        psgn = ctx.enter_context(tc.tile_pool(name="psgn", bufs=2, space="PSUM"))

        # ---- constants, loaded once ----
        def load_const(tag, src, shape, dt=F32):
            t = consts.tile(shape, dt, tag=tag)
            nc.sync.dma_start(out=t, in_=src)
            return t

        wq_sb = [load_const(f"wq{ci}", wq[ci], [128, C], F32R) for ci in range(CT)]
        wk_sb = [load_const(f"wk{ci}", wk[ci], [128, C], F32R) for ci in range(CT)]
        wv_sb = [load_const(f"wv{ci}", wv[ci], [128, C], F32R) for ci in range(CT)]
        wp_sb = [load_const(f"wp{ci}", wp[ci], [128, C], F32R) for ci in range(CT)]
        gnw_sb = [load_const(f"gnw{ci}", gnw[ci], [128, 1]) for ci in range(CT)]
        gnb_sb = [load_const(f"gnb{ci}", gnb[ci], [128, 1]) for ci in range(CT)]
        ones_sb = load_const("ones", ones_d, [128, 128], F32R)
        gmat_sb = load_const("gmat", gmat_d, [128, 128])
        qb_sb = [load_const(f"qb{ci}", qb_d[ci], [128, 1]) for ci in range(CT)] if qb_d is not None else None
        kb_sb = [load_const(f"kb{ci}", kb_d[ci], [128, 1]) for ci in range(CT)] if kb_d is not None else None
        vb_sb = load_const("vb", vb_d, [128, C]) if vb_d is not None else None
        pb_sb = [load_const(f"pb{ci}", pb_d[ci], [128, 1]) for ci in range(CT)] if pb_d is not None else None

        eps_sb = consts.tile([128, 1], F32, tag="eps")
        nc.vector.memset(eps_sb, EPS)

        def body():
          for b in range(BPC):
            # ---- load x ----
            xt = []
            for t in range(CT):
                a = xin.tile([128, N], F32, tag="xt")
                nc.sync.dma_start(out=a, in_=xs[b, t])
                xt.append(a)

            # ---- GroupNorm ----
            ht = []
            for t in range(CT):
                pst = small.tile([128, 2], F32, tag="pst")
                nc.vector.reduce_sum(pst[:, 0:1], xt[t], mybir.AxisListType.X)
                sq = scr.tile([128, N], F32, tag="scr")
                nc.scalar.activation(sq, xt[t], Act.Square,
                                     accum_out=pst[:, 1:2])
                gps = psgn.tile([128, 2], F32, tag="gn")
                # block-diag averaging matrix: group-reduce + broadcast in one
                # matmul; output [128,2] = [mean, E[x^2]] per channel partition
                nc.tensor.matmul(gps, lhsT=gmat_sb, rhs=pst, start=True, stop=True)
                gst = small.tile([128, 2], F32, tag="gst")
                nc.vector.tensor_copy(out=gst, in_=gps)
                tmp = small.tile([128, 8], F32, tag="tmp")
                nc.vector.tensor_tensor(tmp[:, 0:1], gst[:, 0:1], gst[:, 0:1], AluOpType.mult)       # mean^2
                nc.vector.tensor_tensor(tmp[:, 1:2], gst[:, 1:2], tmp[:, 0:1], AluOpType.subtract)   # var
                nc.vector.tensor_scalar_add(tmp[:, 2:3], tmp[:, 1:2], EPS)                           # var+eps
                nc.vector.reciprocal(out=tmp[:, 7:8], in_=tmp[:, 2:3])                               # 1/(var+eps)
                # Sqrt (set 3) + Exp (set 0) keep ACT table switches at 2/batch;
                # Ln would ping-pong sets every GN tile (17 table loads measured)
                nc.scalar.activation(tmp[:, 3:4], tmp[:, 7:8], Act.Sqrt)                             # rstd
                nc.vector.tensor_tensor(tmp[:, 4:5], tmp[:, 3:4], gnw_sb[t], AluOpType.mult)         # s = rstd*w
                nc.vector.tensor_tensor(tmp[:, 5:6], gst[:, 0:1], tmp[:, 4:5], AluOpType.mult)       # mean*s
                nc.vector.tensor_tensor(tmp[:, 6:7], gnb_sb[t], tmp[:, 5:6], AluOpType.subtract)     # b' = b - mean*s
                h_t = hpool.tile([128, N], F32R, tag="ht")
                nc.vector.tensor_scalar(h_t, xt[t], tmp[:, 4:5], tmp[:, 6:7],
                                        op0=AluOpType.mult, op1=AluOpType.add)
                ht.append(h_t)

            # ---- Q, K projections: [d_part, n] ----
            qs = [[None] * NCH for _ in range(CT)]
            ks = []
            for dt_ in range(CT):
                dsl = slice(128 * dt_, 128 * (dt_ + 1))
                k_t = kpool.tile([128, N], F32R, tag="k")
                for nch in range(NCH):
                    nsl = slice(512 * nch, 512 * (nch + 1))
                    ps = psmain.tile([128, 512], F32, tag="ps")
                    for ci in range(CT):
                        nc.tensor.matmul(ps, lhsT=wq_sb[ci][:, dsl],
                                         rhs=ht[ci][:, nsl],
                                         start=(ci == 0), stop=(ci == CT - 1))
                    q_t = qpool.tile([128, 512], F32R, tag="q")
                    if qb_sb is not None:
                        nc.scalar.activation(q_t, ps, Act.Identity, bias=qb_sb[dt_])
                    else:
                        nc.vector.tensor_copy(out=q_t, in_=ps)
                    qs[dt_][nch] = q_t

                    ps2 = psmain.tile([128, 512], F32, tag="ps")
                    for ci in range(CT):
                        nc.tensor.matmul(ps2, lhsT=wk_sb[ci][:, dsl],
                                         rhs=ht[ci][:, nsl],
                                         start=(ci == 0), stop=(ci == CT - 1))
                    if kb_sb is not None:
                        nc.scalar.activation(k_t[:, nsl], ps2, Act.Identity, bias=kb_sb[dt_])
                    else:
                        nc.vector.tensor_copy(out=k_t[:, nsl], in_=ps2)
                ks.append(k_t)

            # ---- V transposed: vT[n_part, c] via h as stationary operand ----
            vts = []
            for nt_ in range(NT):
                psl = slice(128 * nt_, 128 * (nt_ + 1))
                ps = psmain.tile([128, 512], F32, tag="ps")
                for ci in range(CT):
                    nc.tensor.matmul(ps, lhsT=ht[ci][:, psl], rhs=wv_sb[ci],
                                     start=(ci == 0), stop=(ci == CT - 1))
                v_t = vpool.tile([128, 512], F32R, tag="vt")
                if vb_sb is not None:
                    nc.vector.tensor_tensor(v_t, ps, vb_sb, AluOpType.add)
                else:
                    nc.vector.tensor_copy(out=v_t, in_=ps)
                vts.append(v_t)

            # ---- scoresT + exp (softmax without max-subtraction: logits ~N(0,1)) ----
            ats = []
            for pt in range(NT):
                ksl = slice(128 * pt, 128 * (pt + 1))
                a_t = apool.tile([128, N], F32R, tag="at")
                for nch in range(NCH):
                    qsl = slice(512 * nch, 512 * (nch + 1))
                    ps = psmain.tile([128, 512], F32, tag="ps")
                    for ci in range(CT):
                        nc.tensor.matmul(ps, lhsT=ks[ci][:, ksl],
                                         rhs=qs[ci][nch],
                                         start=(ci == 0), stop=(ci == CT - 1))
                    nc.scalar.activation(a_t[:, qsl], ps, Act.Exp, scale=ATTN_SCALE)
                ats.append(a_t)

            # ---- softmax denominator: ones-matmul -> denom broadcast on all
            # partitions; reciprocal once per 512-chunk ----
            recips = []
            for nch in range(NCH):
                qsl = slice(512 * nch, 512 * (nch + 1))
                ps = psmain.tile([128, 512], F32, tag="ps")
                for pt in range(NT):
                    nc.tensor.matmul(ps, lhsT=ones_sb, rhs=ats[pt][:, qsl],
                                     start=(pt == 0), stop=(pt == NT - 1))
                rc = rpool.tile([128, 512], F32, tag="rc")
                nc.vector.reciprocal(out=rc, in_=ps)
                recips.append(rc)

            # ---- AV: h2[c_part, p] with vT as stationary; normalize on evacuation ----
            h2 = []
            for ct_ in range(CT):
                csl = slice(128 * ct_, 128 * (ct_ + 1))
                h2_t = h2pool.tile([128, N], F32R, tag="h2")
                for nch in range(NCH):
                    qsl = slice(512 * nch, 512 * (nch + 1))
                    ps = psmain.tile([128, 512], F32, tag="ps")
                    for pt in range(NT):
                        nc.tensor.matmul(ps, lhsT=vts[pt][:, csl],
                                         rhs=ats[pt][:, qsl],
                                         start=(pt == 0), stop=(pt == NT - 1))
                    nc.vector.tensor_tensor(h2_t[:, qsl], ps, recips[nch], AluOpType.mult)
                h2.append(h2_t)

            # ---- proj + residual (in place into xt) + store ----
            for dt_ in range(CT):
                dsl = slice(128 * dt_, 128 * (dt_ + 1))
                for nch in range(NCH):
                    qsl = slice(512 * nch, 512 * (nch + 1))
                    ps = psmain.tile([128, 512], F32, tag="ps")
                    for ci in range(CT):
                        nc.tensor.matmul(ps, lhsT=wp_sb[ci][:, dsl],
                                         rhs=h2[ci][:, qsl],
                                         start=(ci == 0), stop=(ci == CT - 1))
                    pb_arg = pb_sb[dt_] if pb_sb is not None else 0.0
                    nc.vector.scalar_tensor_tensor(
                        out=xt[dt_][:, qsl], in0=ps, scalar=pb_arg,
                        in1=xt[dt_][:, qsl],
                        op0=AluOpType.add, op1=AluOpType.add)
                nc.sync.dma_start(out=out_d[b, dt_], in_=xt[dt_])

        if loop_reps is None:
            body()
        else:
            with tc.For_i(0, loop_reps, 1):
                body()


def _prep_inputs(x, gn_w, gn_b, q_w, q_b, k_w, k_b, v_w, v_b, p_w, p_b):
    f = np.float32
    x = np.ascontiguousarray(np.asarray(x, f)).reshape(B, CT, 128, N)
    base = {
        "wqT": np.ascontiguousarray(np.asarray(q_w, f).T).reshape(CT, 128, C),
        "wkT": np.ascontiguousarray(np.asarray(k_w, f).T).reshape(CT, 128, C),
        "wvT": np.ascontiguousarray(np.asarray(v_w, f).T).reshape(CT, 128, C),
        "wpT": np.ascontiguousarray(np.asarray(p_w, f).T).reshape(CT, 128, C),
        "gnw": np.ascontiguousarray(np.asarray(gn_w, f)).reshape(CT, 128, 1),
        "gnb": np.ascontiguousarray(np.asarray(gn_b, f)).reshape(CT, 128, 1),
        "ones": np.ones((128, 128), f),
        # block-diagonal group-averaging matrix, scaled so the matmul yields
        # means directly: G[p, m] = 1/(GS*N) iff p//GS == m//GS
        "gmat": np.ascontiguousarray(
            np.kron(np.eye(128 // GS, dtype=f), np.ones((GS, GS), f)) / (GS * N)),
    }
    flags = tuple(bool(np.any(np.asarray(v))) for v in (q_b, k_b, v_b, p_b))
    qb_nz, kb_nz, vb_nz, pb_nz = flags
    if qb_nz:
        base["qb"] = np.ascontiguousarray(np.asarray(q_b, f)).reshape(CT, 128, 1)
    if kb_nz:
        base["kb"] = np.ascontiguousarray(np.asarray(k_b, f)).reshape(CT, 128, 1)
    if vb_nz:
        base["vb"] = np.ascontiguousarray(
            np.broadcast_to(np.asarray(v_b, f)[None, :], (128, C)).copy())
    if pb_nz:
        base["pb"] = np.ascontiguousarray(np.asarray(p_b, f)).reshape(CT, 128, 1)
    return x, base, flags


def kernel(x, temb, gn_w, gn_b, q_w, q_b, k_w, k_b, v_w, v_b, p_w, p_b):
    global LAST_RESULTS
    del temb  # unused by the reference module
    x_r, base, flags = _prep_inputs(x, gn_w, gn_b, q_w, q_b, k_w, k_b,
                                    v_w, v_b, p_w, p_b)
    if flags not in _PROGRAM_CACHE:
        _PROGRAM_CACHE[flags] = _build_program(flags)
    nc = _PROGRAM_CACHE[flags]

    in_maps = [dict(base, xs=np.ascontiguousarray(x_r[BPC * i: BPC * (i + 1)]))
               for i in range(NCORES)]
    res = run_bass_kernel_spmd(nc, in_maps, core_ids=list(range(NCORES)))
    LAST_RESULTS = res
    out = np.concatenate([r["out"] for r in res.results], axis=0)
    return np.ascontiguousarray(out.reshape(B, C, H, W).astype(np.float32))
